# revision 1
# baseline (speedup 1.0000x reference)
"""MoE top-2 routing kernel for 8 TRN2 NeuronCores (expert-parallel).

Strategy: each core c owns expert c (E == n_cores == 8).
 - Router is replicated: every core computes logits/softmax/top-2 for all
   N=8192 tokens in fp32 (PE matmul + PE transpose + DVE softmax).
 - Each core compacts the token list routed to its expert on device
   (sparse_gather), gathers those token rows (dma_gather transpose),
   runs the expert FFN in bf16, scales by the gate, and scatter-adds
   into a per-core partial output (8192, 512).
 - Host-side unshard: sum the 8 partial outputs (each token appears on
   exactly its top-2 expert cores).
No collectives needed.
"""

import os
import numpy as np

B, S, D, H, E = 4, 2048, 512, 1024, 8
N = B * S                      # 8192 tokens
CAP = 2560                     # per-expert token capacity (max observed 2192)
KD = D // 128                  # 4 contraction chunks over D
KH = H // 128                  # 8 contraction chunks over H
MB = H // 128                  # 8 output blocks for fc1
NB = CAP // 512                # 5 moving blocks of 512 tokens for fc1
TB = CAP // 128                # 20 token blocks for fc2
NT = N // 128                  # 64 token tiles
RNB = N // 512                 # 16 router matmul blocks
CAPI = CAP // 16               # 160 idx columns

_cached = None


def build_nc(debug_outs: bool = False, stage: int = 4):
    """stage: 1=router+compaction, 2=+gather, 3=+ffn, 4=full (scatter)."""
    import concourse.bass as bass
    import concourse.bacc as bacc
    import concourse.mybir as mybir
    from concourse import tile

    f32 = mybir.dt.float32
    bf16 = mybir.dt.bfloat16
    i16 = mybir.dt.int16
    u32 = mybir.dt.uint32
    AF = mybir.ActivationFunctionType
    OP = mybir.AluOpType
    AX = mybir.AxisListType

    nc = bacc.Bacc("TRN2", target_bir_lowering=False, debug=False,
                   num_devices=8)

    # ---- DRAM I/O ----
    xt_d = nc.dram_tensor("xt", [KD, 128, N], f32, kind="ExternalInput")
    xrow_d = nc.dram_tensor("xrow", [N, D], bf16, kind="ExternalInput")
    wrt_d = nc.dram_tensor("wrt", [KD, 128, E], f32, kind="ExternalInput")
    brc_d = nc.dram_tensor("brc", [E, 1], f32, kind="ExternalInput")
    sel_d = nc.dram_tensor("sel", [128, E], f32, kind="ExternalInput")
    ident_d = nc.dram_tensor("ident", [128, 128], f32, kind="ExternalInput")
    w1_d = nc.dram_tensor("w1", [KD, 128, H], bf16, kind="ExternalInput")
    b1t_d = nc.dram_tensor("b1t", [128, MB], f32, kind="ExternalInput")
    w2_d = nc.dram_tensor("w2", [KH, 128, D], bf16, kind="ExternalInput")
    b2r_d = nc.dram_tensor("b2r", [1, D], bf16, kind="ExternalInput")
    y_d = nc.dram_tensor("y", [N, D], bf16, kind="ExternalOutput")
    if stage < 4:
        debug_outs = True
        dbg_xg_d = nc.dram_tensor("dbg_xg", [128, KD, 128], bf16,
                                  kind="ExternalOutput")
        dbg_out_d = nc.dram_tensor("dbg_out", [128, 2, D], bf16,
                                   kind="ExternalOutput")
    if debug_outs:
        dbg_gates_d = nc.dram_tensor("dbg_gates", [128, NT], f32,
                                     kind="ExternalOutput")
        dbg_idx_d = nc.dram_tensor("dbg_idx", [16, CAPI], i16,
                                   kind="ExternalOutput")
        dbg_cnt_d = nc.dram_tensor("dbg_cnt", [1, 1], u32,
                                   kind="ExternalOutput")
        dbg_gsel_d = nc.dram_tensor("dbg_gsel", [16, CAPI], f32,
                                    kind="ExternalOutput")

    with tile.TileContext(nc) as tc:
        from contextlib import ExitStack
        with (
            tc.tile_pool(name="consts", bufs=1) as cpool,
            tc.tile_pool(name="xtiles", bufs=3) as xpool,
            tc.tile_pool(name="lgs", bufs=2) as lgs,
            tc.tile_pool(name="soft", bufs=3) as soft,
            tc.tile_pool(name="comp", bufs=1) as comp,
            tc.tile_pool(name="big", bufs=1) as big,
            ExitStack() as psum_stack,
            ExitStack() as fc_stack,
        ):
            lgp = psum_stack.enter_context(
                tc.tile_pool(name="lgp", bufs=2, space=bass.MemorySpace.PSUM))
            trp = psum_stack.enter_context(
                tc.tile_pool(name="trp", bufs=1, space=bass.MemorySpace.PSUM))
            # ---- constants into SBUF ----
            wrt_sb = cpool.tile([128, KD * E], f32)
            for k in range(KD):
                nc.sync.dma_start(wrt_sb[:, k * E:(k + 1) * E], wrt_d[k])
            br_sb = cpool.tile([E, 1], f32)
            nc.sync.dma_start(br_sb[:], brc_d[:, :])
            sel_sb = cpool.tile([128, E], f32)
            nc.sync.dma_start(sel_sb[:], sel_d[:, :])
            ident_sb = cpool.tile([128, 128], f32)
            nc.sync.dma_start(ident_sb[:], ident_d[:, :])
            w1_sb = cpool.tile([128, KD * H], bf16)
            for k in range(KD):
                nc.sync.dma_start(w1_sb[:, k * H:(k + 1) * H], w1_d[k])
            b1_sb = cpool.tile([128, MB], f32)
            nc.sync.dma_start(b1_sb[:], b1t_d[:, :])
            w2_sb = cpool.tile([128, KH, D], bf16)
            for k in range(KH):
                nc.sync.dma_start(w2_sb[:, k, :], w2_d[k])
            b2_sb = cpool.tile([1, D], bf16)
            nc.sync.dma_start(b2_sb[:], b2r_d[:, :])
            ones_sb = cpool.tile([1, 128], bf16)
            nc.vector.memset(ones_sb[:], 1.0)

            # big tiles
            # gathered x^T, in 512-slot chunks (single big SWDGE gathers
            # crash the device; chunks also let fc1 start per-chunk)
            xg_chunks = [big.tile([128, KD, 512], bf16, name=f"xg{j}")
                         for j in range(NB)]
            h_sb = big.tile([128, KH, CAP], bf16)     # fc1 output (H on parts)
            out_sb = big.tile([128, TB, D], bf16)     # gated fc2 output

            # ---- router: logitsT (E, N) in fp32, transposed to (tok, E) ----
            tr = trp.tile([128, NT, E], f32)          # logits, token-major
            g_all = soft.tile([128, NT], f32)         # this-core gate per token
            for nb in range(RNB):
                xt_t = xpool.tile([128, KD, 512], f32, tag="xt")
                nc.sync.dma_start(
                    xt_t[:],
                    xt_d[:, :, nb * 512:(nb + 1) * 512].rearrange(
                        "k p t -> p k t"),
                )
                lg = lgp.tile([E, 512], f32)
                for k in range(KD):
                    nc.tensor.matmul(
                        lg[:],
                        wrt_sb[:, k * E:(k + 1) * E],
                        xt_t[:, k, :],
                        start=(k == 0),
                        stop=(k == KD - 1),
                    )
                lgt = lgs.tile([E, 512], f32)
                # PSUM -> SBUF copy, adding router bias per expert row
                nc.scalar.activation(lgt[:], lg[:], AF.Identity,
                                     bias=br_sb[:, 0:1], scale=1.0)
                for jj in range(4):
                    j = nb * 4 + jj
                    nc.tensor.transpose(
                        tr[:, j, :],
                        lgt[:, jj * 128:(jj + 1) * 128],
                        ident_sb[:E, :E],
                    )
                # per-block softmax + top2 + this-core gate: overlaps the
                # remaining router matmuls instead of trailing them
                NB4 = 4
                trb = tr[:, nb * 4:(nb + 1) * 4, :]
                m1 = soft.tile([128, NB4], f32, tag="m1")
                nc.vector.tensor_reduce(m1[:], trb, axis=AX.X, op=OP.max)
                lm1 = soft.tile([128, NB4, E], f32, tag="lm1")
                nc.vector.tensor_tensor(lm1[:], trb,
                                        m1[:].broadcast_to([128, NB4, E]),
                                        op=OP.subtract)
                e_l = soft.tile([128, NB4, E], f32, tag="e_l")
                nc.scalar.activation(e_l[:], lm1[:], AF.Exp)
                zs = soft.tile([128, NB4], f32, tag="zs")
                nc.vector.tensor_reduce(zs[:], e_l[:], axis=AX.X, op=OP.add)
                mask1 = soft.tile([128, NB4, E], f32, tag="mask1")
                nc.vector.tensor_tensor(mask1[:], trb,
                                        m1[:].broadcast_to([128, NB4, E]),
                                        op=OP.is_ge)
                lm = soft.tile([128, NB4, E], f32, tag="lm")
                nc.vector.scalar_tensor_tensor(lm[:], mask1[:], -1e30, trb,
                                               op0=OP.mult, op1=OP.add)
                m2 = soft.tile([128, NB4], f32, tag="m2")
                nc.vector.tensor_reduce(m2[:], lm[:], axis=AX.X, op=OP.max)
                mask2 = soft.tile([128, NB4, E], f32, tag="mask2")
                nc.vector.tensor_tensor(mask2[:], trb,
                                        m2[:].broadcast_to([128, NB4, E]),
                                        op=OP.is_ge)
                gnum_t = soft.tile([128, NB4, E], f32, tag="gnum_t")
                nc.vector.tensor_tensor(gnum_t[:], e_l[:], mask2[:],
                                        op=OP.mult)
                gsel_t = soft.tile([128, NB4, E], f32, tag="gsel_t")
                nc.vector.tensor_tensor(
                    gsel_t[:], gnum_t[:],
                    sel_sb[:, None, :].broadcast_to([128, NB4, E]),
                    op=OP.mult)
                gnum = soft.tile([128, NB4], f32, tag="gnum")
                nc.vector.tensor_reduce(gnum[:], gsel_t[:], axis=AX.X,
                                        op=OP.add)
                rz = soft.tile([128, NB4], f32, tag="rz")
                nc.vector.reciprocal(rz[:], zs[:])
                nc.vector.tensor_tensor(g_all[:, nb * 4:(nb + 1) * 4],
                                        gnum[:], rz[:], op=OP.mult)
            if debug_outs:
                nc.sync.dma_start(dbg_gates_d[:, :], g_all[:])
            psum_stack.close()  # release router PSUM banks for the FFN
            fc1p = fc_stack.enter_context(
                tc.tile_pool(name="fc1p", bufs=5, space=bass.MemorySpace.PSUM))
            fc2p = fc_stack.enter_context(
                tc.tile_pool(name="fc2p", bufs=3, space=bass.MemorySpace.PSUM))

            # ---- compaction: build slot -> token idx + gate lists ----
            g16 = comp.tile([16, N // 16], f32)
            for a in range(8):
                nc.sync.dma_start(
                    g16[:, a::8].rearrange("p (o t) -> p o t", o=1),
                    g_all[16 * a:16 * (a + 1), None, :],
                )
            mask16 = comp.tile([16, N // 16], mybir.dt.uint8)
            nc.vector.tensor_single_scalar(mask16[:], g16[:], 0.0, op=OP.is_gt)
            iota_i = comp.tile([16, N // 16], mybir.dt.int32)
            nc.gpsimd.iota(iota_i[:], pattern=[[16, N // 16]], base=0,
                           channel_multiplier=1)
            iota_t = comp.tile([16, N // 16], f32)
            nc.vector.tensor_copy(iota_t[:], iota_i[:])
            neg1 = comp.tile([16, N // 16], f32)
            nc.vector.memset(neg1[:], -1.0)
            # pack token id + gate/2 into one value -> single sparse_gather;
            # gate/2 < 0.5 so the packed sum never rounds to the next integer
            pack = comp.tile([16, N // 16], f32)
            nc.vector.scalar_tensor_tensor(pack[:], g16[:], 0.5, iota_t[:],
                                           op0=OP.mult, op1=OP.add)
            tokv = comp.tile([16, N // 16], f32)
            nc.vector.select(tokv[:], mask16[:], pack[:], neg1[:])

            cmb_cmp = comp.tile([16, CAPI], f32)
            nf = comp.tile([1, 1], u32)
            nc.gpsimd.sparse_gather(cmb_cmp[:], tokv[:], num_found=nf[:])

            nf_f = comp.tile([1, 1], f32)
            nc.vector.tensor_copy(nf_f[:], nf[:])
            nf_b = comp.tile([16, 1], f32)
            nc.gpsimd.partition_broadcast(nf_b[:], nf_f[:])
            slot_i = comp.tile([16, CAPI], mybir.dt.int32)
            nc.gpsimd.iota(slot_i[:], pattern=[[16, CAPI]], base=0,
                           channel_multiplier=1)
            slot_io = comp.tile([16, CAPI], f32)
            nc.vector.tensor_copy(slot_io[:], slot_i[:])
            padm = comp.tile([16, CAPI], mybir.dt.uint8)
            nc.vector.tensor_tensor(padm[:], slot_io[:],
                                    nf_b[:].broadcast_to([16, CAPI]),
                                    op=OP.is_lt)
            # Pad slots use token 0 with gate 0: the scatter then adds an
            # exact 0.0 row to token 0 (numeric no-op), so every slot is
            # valid and the SWDGE count is the compile-time constant CAP.
            zero16 = comp.tile([16, CAPI], f32)
            nc.vector.memset(zero16[:], 0.0)
            idx_f = comp.tile([16, CAPI], f32)
            nc.vector.select(idx_f[:], padm[:], cmb_cmp[:], zero16[:])
            # f32->int16 truncation recovers the token id (frac = gate/2 < .5)
            idx16 = comp.tile([16, CAPI], i16)
            nc.vector.tensor_copy(idx16[:], idx_f[:])
            tokf = comp.tile([16, CAPI], f32)
            nc.vector.tensor_copy(tokf[:], idx16[:])
            gates_c = comp.tile([16, CAPI], f32)
            nc.vector.tensor_tensor(gates_c[:], idx_f[:], tokf[:],
                                    op=OP.subtract)
            if debug_outs:
                nc.sync.dma_start(dbg_idx_d[:, :], idx16[:])
                nc.sync.dma_start(dbg_cnt_d[:, :], nf[:])
                nc.sync.dma_start(dbg_gsel_d[:, :], gates_c[:])

            idx128 = comp.tile([128, CAPI], i16)
            for r in range(8):
                nc.sync.dma_start(idx128[16 * r:16 * (r + 1), :], idx16[:])
            gate_cols = comp.tile([128, TB], f32)
            for r in range(8):
                nc.sync.dma_start(
                    gate_cols[16 * r:16 * (r + 1), None, :],
                    gates_c[:, r::8].rearrange("p (o t) -> p o t", o=1),
                )

            # ---- gather selected token rows (transposed into xg) ----
            if stage >= 2:
                for j in range(NB):
                    nc.gpsimd.dma_gather(
                        xg_chunks[j][:], xrow_d[:, :],
                        idx128[:, j * 32:(j + 1) * 32],
                        num_idxs=512, num_idxs_reg=512, elem_size=D,
                        transpose=True,
                    )
            if stage == 2:
                nc.sync.dma_start(dbg_xg_d[:, :, :], xg_chunks[0][:, :, 0:128])

            # ---- fc1: hT[m] = gelu(W1[:,m]^T @ xg + b1[m]) ----
            # n-block outer: h columns for a 512-token chunk finish together,
            # so fc2 t-blocks start while later fc1 chunks still run (keeps
            # the PE dense and HAM-warm)
            for n in range(NB if stage >= 3 else 0):
                for m in range(MB):
                    ps = fc1p.tile([128, 512], f32, tag="fc1ps",
                                   name=f"fc1ps_{n}_{m}")
                    for k in range(KD):
                        lhs = w1_sb[:, k * H + m * 128: k * H + (m + 1) * 128]
                        nc.tensor.matmul(
                            ps[:], lhs, xg_chunks[n][:, k, :],
                            start=(k == 0), stop=(k == KD - 1),
                        )
                    nc.scalar.activation(
                        h_sb[:, m, n * 512:(n + 1) * 512], ps[:],
                        AF.Gelu, bias=b1_sb[:, m:m + 1], scale=1.0)

            # ---- fc2: out[t] = (hT[:,t]^T @ W2 + b2) * gate ----
            for t in range(TB if stage >= 3 else 0):
                po = fc2p.tile([128, D], f32, tag="fc2ps")
                for k in range(KH):
                    nc.tensor.matmul(
                        po[:], h_sb[:, k, t * 128:(t + 1) * 128], w2_sb[:, k, :],
                        start=(k == 0), stop=False,
                    )
                nc.tensor.matmul(po[:], ones_sb[:, :], b2_sb[:, :],
                                 start=False, stop=True)
                # gate_cols holds gate/2 (packed-compaction); x2 restores it
                nc.vector.tensor_scalar(out_sb[:, t, :], po[:],
                                        gate_cols[:, t:t + 1], 2.0,
                                        op0=OP.mult, op1=OP.mult)

            # ---- scatter-add into the (pre-zeroed) partial output ----
            if stage == 3:
                nc.sync.dma_start(dbg_out_d[:, :, :], out_sb[:, 0:2, :])
            if stage >= 4:
                for j in range(NB):
                    nc.gpsimd.dma_scatter_add(
                        y_d[:, :], out_sb[:, 4 * j:4 * (j + 1), :],
                        idx128[:, j * 32:(j + 1) * 32],
                        num_idxs=512, num_idxs_reg=512, elem_size=D,
                    )

    nc.compile()
    return nc


def get_nc(debug_outs: bool = False):
    global _cached
    if _cached is None or _cached[1] != debug_outs:
        _cached = (build_nc(debug_outs), debug_outs)
    return _cached[0]


def make_in_maps(inputs):
    import concourse.mybir as mybir
    bf16 = mybir.dt.np(mybir.dt.bfloat16)

    x = np.asarray(inputs["x"], np.float32)
    Wr = np.asarray(inputs["Wr"], np.float32)
    br = np.asarray(inputs["br"], np.float32)
    W1 = np.asarray(inputs["W1"], np.float32)
    b1 = np.asarray(inputs["b1"], np.float32)
    W2 = np.asarray(inputs["W2"], np.float32)
    b2 = np.asarray(inputs["b2"], np.float32)

    xf = np.ascontiguousarray(x.reshape(N, D))
    xt = np.ascontiguousarray(xf.T).reshape(KD, 128, N)
    xrow = xf.astype(bf16)
    wrt = np.ascontiguousarray(Wr.T).reshape(KD, 128, E)
    brc = np.ascontiguousarray(br.reshape(E, 1))
    ident = np.eye(128, dtype=np.float32)

    in_maps = []
    for c in range(E):
        sel = np.zeros((128, E), np.float32)
        sel[:, c] = 1.0
        in_maps.append({
            "xt": xt,
            "xrow": xrow,
            "wrt": wrt,
            "brc": brc,
            "sel": sel,
            "ident": ident,
            "w1": np.ascontiguousarray(W1[c]).astype(bf16).reshape(KD, 128, H),
            "b1t": np.ascontiguousarray(b1[c].reshape(MB, 128).T),
            "w2": np.ascontiguousarray(W2[c]).astype(bf16).reshape(KH, 128, D),
            "b2r": b2[c].reshape(1, D).astype(bf16),
        })
    return in_maps


last_results = None


def _ensure_ntff_hook():
    """Register the axon NTFF profile hook when antenv.axon_hooks is absent."""
    import sys, types
    try:
        from antenv.axon_hooks import get_axon_ntff_profile_hook  # noqa: F401
        return True
    except ImportError:
        pass
    try:
        mod = types.ModuleType("antenv.axon_hooks")
        mod._hook = None
        mod.set_axon_ntff_profile_hook = lambda h: setattr(mod, "_hook", h)
        mod.get_axon_ntff_profile_hook = lambda: mod._hook
        sys.modules["antenv.axon_hooks"] = mod
        import antenv
        antenv.axon_hooks = mod
        from trn_agent_boot.trn_boot import _ntff_profile_via_ctypes
        mod._hook = _ntff_profile_via_ctypes("/opt/axon/libaxon_pjrt.so")
        return mod._hook is not None
    except Exception as e:  # profiling is best-effort
        print(f"ntff hook setup failed: {e}")
        return False


def kernel(**inputs):
    global last_results
    from concourse import bass_utils

    nc = get_nc()
    in_maps = make_in_maps(inputs)
    trace = bool(int(os.environ.get("MOE_TRACE", "0")))
    kwargs = {}
    if trace and _ensure_ntff_hook():
        kwargs = dict(trace=True, trace_cores=list(range(E)))
    res = bass_utils.run_bass_kernel_spmd(nc, in_maps,
                                          core_ids=list(range(E)), **kwargs)
    last_results = res
    y = np.zeros((N, D), np.float32)
    for c in range(E):
        y += np.asarray(res.results[c]["y"], dtype=np.float32)
    return y.reshape(B, S, D)

